# revision 1
# baseline (speedup 1.0000x reference)
"""InfoNCE lower-bound kernel for 8 Trainium2 NeuronCores.

Math (reference):
  hx = x @ W1x.T ; hy = y @ W1y.T            [N, H]
  z_ij = relu(hx[j] + hy[i] + b1) . w2       (logit WITHOUT b2)
  T1[i,j] = softplus(z_ij + b2)
  T0[i]   = T1[i,i]
  lse[i]  = log(sum_j exp(T1[i,j]))
  out     = mean(T0) - (mean(lse) - log N)

Key identity used on-device:  exp(softplus(v)) = 1 + e^v, so
  lse[i] = log(N + sum_j exp(z_ij + b2))
which avoids materializing softplus over the [N, N] grid.

Sharding: data-parallel over i (rows of the pair grid). Each of the 8
cores gets 64 rows (its slice of y), x and the MLP params replicated.
Per-core partial sums of T0 and lse are combined on the host.
"""

import math

import numpy as np

N = 512
XD = 768
YD = 768
H = 300
NCORES = 8
ISH = N // NCORES  # 64 rows per core
KD = XD // 128     # 6 contraction tiles of 128
HT = 3             # h tiles: 128, 128, 44
HSZ = [128, 128, H - 256]

_CACHE = {}
TRACE = False
LAST_RESULTS = None


def _build_module():
    import concourse.bacc as bacc
    import concourse.mybir as mybir
    from concourse.tile import TileContext

    f32 = mybir.dt.float32
    AF = mybir.ActivationFunctionType
    ALU = mybir.AluOpType
    AX = mybir.AxisListType

    nc = bacc.Bacc("TRN2", target_bir_lowering=False, debug=False)

    # Per-core inputs (SPMD: same shapes, different data for yT/xTd).
    xT = nc.dram_tensor("xT", [XD, N], f32, kind="ExternalInput")        # x^T
    w1xT = nc.dram_tensor("w1xT", [XD, H], f32, kind="ExternalInput")    # W1x^T
    w1yT = nc.dram_tensor("w1yT", [YD, H], f32, kind="ExternalInput")    # W1y^T
    yT = nc.dram_tensor("yT", [YD, ISH], f32, kind="ExternalInput")      # y-slice^T
    xTd = nc.dram_tensor("xTd", [XD, ISH], f32, kind="ExternalInput")    # x-slice^T (diag)
    b1p = nc.dram_tensor("b1p", [128, HT], f32, kind="ExternalInput")    # b1 packed
    w2p = nc.dram_tensor("w2p", [128, HT], f32, kind="ExternalInput")    # w2 packed
    b2r = nc.dram_tensor("b2r", [128, 1], f32, kind="ExternalInput")     # b2 replicated
    out = nc.dram_tensor("out", [1, 2], f32, kind="ExternalOutput")      # [t0_sum, lse_sum]

    with TileContext(nc) as tc:
        cpool = tc.alloc_tile_pool(name="consts", bufs=1)
        rpool = tc.alloc_tile_pool(name="work", bufs=6)
        tpool = tc.alloc_tile_pool(name="tail", bufs=1)
        pp_pre = tc.alloc_tile_pool(name="pp_pre", bufs=2, space="PSUM")
        pp_z = tc.alloc_tile_pool(name="pp_z", bufs=3, space="PSUM")
        pp_misc = tc.alloc_tile_pool(name="pp_misc", bufs=1, space="PSUM")

        # ---- load constants / inputs into SBUF ----
        xt_sb = cpool.tile([128, KD * N], f32, tag="xt")
        w1x_sb = cpool.tile([128, KD * H], f32, tag="w1x")
        w1y_sb = cpool.tile([128, KD * H], f32, tag="w1y")
        yt_sb = cpool.tile([128, KD * ISH], f32, tag="yt")
        xtd_sb = cpool.tile([128, KD * ISH], f32, tag="xtd")
        b1_sb = cpool.tile([128, HT], f32, tag="b1")
        w2_sb = cpool.tile([128, HT], f32, tag="w2")
        b2_sb = cpool.tile([128, 1], f32, tag="b2")

        for k in range(KD):
            nc.sync.dma_start(xt_sb[:, k * N:(k + 1) * N], xT[k * 128:(k + 1) * 128, :])
            nc.sync.dma_start(w1x_sb[:, k * H:(k + 1) * H], w1xT[k * 128:(k + 1) * 128, :])
            nc.sync.dma_start(w1y_sb[:, k * H:(k + 1) * H], w1yT[k * 128:(k + 1) * 128, :])
            nc.sync.dma_start(yt_sb[:, k * ISH:(k + 1) * ISH], yT[k * 128:(k + 1) * 128, :])
            nc.sync.dma_start(xtd_sb[:, k * ISH:(k + 1) * ISH], xTd[k * 128:(k + 1) * 128, :])
        nc.sync.dma_start(b1_sb[:], b1p[:])
        nc.sync.dma_start(w2_sb[:], w2p[:])
        nc.sync.dma_start(b2_sb[:], b2r[:])

        # ---- precompute hxT(+b1), hyT, hxdT on device ----
        hxb_sb = cpool.tile([128, HT * N], f32, tag="hxb")    # relu-arg x part (+b1)
        hy_sb = cpool.tile([128, HT * ISH], f32, tag="hy")    # y part
        hxd_sb = cpool.tile([128, HT * ISH], f32, tag="hxd")  # diag x part (+b1)
        nc.vector.memset(hxb_sb[:, 2 * N:3 * N], 0.0)
        nc.vector.memset(hy_sb[:, 2 * ISH:3 * ISH], 0.0)
        nc.vector.memset(hxd_sb[:, 2 * ISH:3 * ISH], 0.0)

        for t in range(HT):
            hs = HSZ[t]
            ps = pp_pre.tile([128, N], f32, tag="pre")
            for k in range(KD):
                nc.tensor.matmul(
                    ps[0:hs, :],
                    lhsT=w1x_sb[:, k * H + 128 * t: k * H + 128 * t + hs],
                    rhs=xt_sb[:, k * N:(k + 1) * N],
                    start=(k == 0), stop=(k == KD - 1),
                )
            nc.scalar.activation(
                hxb_sb[0:hs, t * N:(t + 1) * N], ps[0:hs, :],
                AF.Identity, bias=b1_sb[0:hs, t:t + 1],
            )

        for t in range(HT):
            hs = HSZ[t]
            psy = pp_pre.tile([128, ISH], f32, tag="pre")
            for k in range(KD):
                nc.tensor.matmul(
                    psy[0:hs, :],
                    lhsT=w1y_sb[:, k * H + 128 * t: k * H + 128 * t + hs],
                    rhs=yt_sb[:, k * ISH:(k + 1) * ISH],
                    start=(k == 0), stop=(k == KD - 1),
                )
            nc.vector.tensor_copy(hy_sb[0:hs, t * ISH:(t + 1) * ISH], psy[0:hs, :])

        for t in range(HT):
            hs = HSZ[t]
            psd = pp_pre.tile([128, ISH], f32, tag="pre")
            for k in range(KD):
                nc.tensor.matmul(
                    psd[0:hs, :],
                    lhsT=w1x_sb[:, k * H + 128 * t: k * H + 128 * t + hs],
                    rhs=xtd_sb[:, k * ISH:(k + 1) * ISH],
                    start=(k == 0), stop=(k == KD - 1),
                )
            nc.scalar.activation(
                hxd_sb[0:hs, t * ISH:(t + 1) * ISH], psd[0:hs, :],
                AF.Identity, bias=b1_sb[0:hs, t:t + 1],
            )

        # ---- main loop: z rows via relu + matvec, 4 rows per PSUM bank ----
        zrows = cpool.tile([ISH, N], f32, tag="zrows")
        for g in range(ISH // 4):
            zp = pp_z.tile([128, N], f32, tag="zp")
            for k4 in range(4):
                i = 4 * g + k4
                for t in range(HT):
                    r = rpool.tile([128, N], f32, tag="r")
                    col = hy_sb[:, t * ISH + i: t * ISH + i + 1]
                    src = hxb_sb[:, t * N:(t + 1) * N]
                    if t == 1:
                        nc.scalar.activation(r[:], src, AF.Relu, bias=col)
                    else:
                        nc.vector.tensor_scalar(r[:], src, col, 0.0, ALU.add, ALU.max)
                    nc.tensor.matmul(
                        zp[32 * k4:32 * k4 + 1, :],
                        lhsT=w2_sb[:, t:t + 1], rhs=r[:],
                        start=(t == 0), stop=(t == HT - 1),
                        tile_position=(0, 32 * k4),
                    )
            zst = rpool.tile([128, N], f32, tag="zst")
            if g % 2 == 0:
                nc.vector.tensor_copy(zst[:], zp[:])
            else:
                nc.scalar.copy(zst[:], zp[:])
            zst_rows = zst[:].rearrange("(a b) f -> a b f", b=32)[:, 0, :]
            nc.sync.dma_start(zrows[4 * g:4 * g + 4, :], zst_rows)

        # ---- tail: lse partial ----
        ee = tpool.tile([ISH, N], f32, tag="ee")
        sexp = tpool.tile([ISH, 1], f32, tag="sexp")
        nc.scalar.activation(ee[:], zrows[:], AF.Exp, bias=b2_sb[0:ISH, 0:1])
        nc.vector.tensor_reduce(sexp[:], ee[:], axis=AX.X, op=ALU.add)
        lsev = tpool.tile([ISH, 1], f32, tag="lsev")
        nconst = tpool.tile([ISH, 1], f32, tag="nconst")
        nc.vector.memset(nconst[:], float(N))
        nc.scalar.activation(lsev[:], sexp[:], AF.Ln, bias=nconst[0:ISH, 0:1])
        onesv = tpool.tile([ISH, 1], f32, tag="ones")
        nc.vector.memset(onesv[:], 1.0)
        lsum_ps = pp_misc.tile([128, 1], f32, tag="lsum")
        nc.tensor.matmul(
            lsum_ps[0:1, 0:1], lhsT=onesv[0:ISH, 0:1], rhs=lsev[0:ISH, 0:1],
            start=True, stop=True,
        )

        # ---- tail: T0 partial from diagonal ----
        dps = pp_misc.tile([128, ISH], f32, tag="dps")
        for t in range(HT):
            dsum = tpool.tile([128, ISH], f32, tag="dsum")
            nc.vector.tensor_add(
                dsum[:], hxd_sb[:, t * ISH:(t + 1) * ISH], hy_sb[:, t * ISH:(t + 1) * ISH]
            )
            dr = tpool.tile([128, ISH], f32, tag="dr")
            nc.vector.tensor_scalar(dr[:], dsum[:], 0.0, None, ALU.max)
            nc.tensor.matmul(
                dps[0:1, :], lhsT=w2_sb[:, t:t + 1], rhs=dr[:],
                start=(t == 0), stop=(t == HT - 1),
            )
        ed = tpool.tile([1, ISH], f32, tag="ed")
        nc.scalar.activation(ed[:], dps[0:1, :], AF.Exp, bias=b2_sb[0:1, 0:1])
        t0v = tpool.tile([1, ISH], f32, tag="t0v")
        nc.scalar.activation(t0v[:], ed[:], AF.Ln, bias=onesv[0:1, 0:1])

        final = tpool.tile([1, 2], f32, tag="final")
        nc.vector.tensor_reduce(final[0:1, 0:1], t0v[0:1, :], axis=AX.X, op=ALU.add)
        nc.scalar.copy(final[0:1, 1:2], lsum_ps[0:1, 0:1])
        nc.sync.dma_start(out[0:1, :], final[0:1, :])

        for p in (pp_misc, pp_z, pp_pre, tpool, rpool, cpool):
            p.release()

    nc.finalize()
    return nc


def _get_module():
    if "nc" not in _CACHE:
        _CACHE["nc"] = _build_module()
    return _CACHE["nc"]


def kernel(**inputs) -> np.ndarray:
    from concourse.bass_utils import run_bass_kernel_spmd

    x = np.ascontiguousarray(np.asarray(inputs["x_samples"], dtype=np.float32))
    y = np.ascontiguousarray(np.asarray(inputs["y_samples"], dtype=np.float32))
    W1 = np.asarray(inputs["W1"], dtype=np.float32)
    b1 = np.asarray(inputs["b1"], dtype=np.float32).reshape(H)
    W2 = np.asarray(inputs["W2"], dtype=np.float32)
    b2 = float(np.asarray(inputs["b2"], dtype=np.float32).reshape(1)[0])

    xT = np.ascontiguousarray(x.T)                      # [768, 512]
    w1xT = np.ascontiguousarray(W1[:, :XD].T)           # [768, 300]
    w1yT = np.ascontiguousarray(W1[:, XD:].T)           # [768, 300]

    b1p = np.zeros((128, HT), np.float32)
    w2p = np.zeros((128, HT), np.float32)
    w2 = W2.reshape(H)
    for t in range(HT):
        hs = HSZ[t]
        b1p[:hs, t] = b1[128 * t:128 * t + hs]
        w2p[:hs, t] = w2[128 * t:128 * t + hs]
    b2r = np.full((128, 1), b2, np.float32)

    in_maps = []
    for c in range(NCORES):
        sl = slice(c * ISH, (c + 1) * ISH)
        in_maps.append({
            "xT": xT,
            "w1xT": w1xT,
            "w1yT": w1yT,
            "yT": np.ascontiguousarray(y[sl].T),        # [768, 64]
            "xTd": np.ascontiguousarray(x[sl].T),       # [768, 64]
            "b1p": b1p,
            "w2p": w2p,
            "b2r": b2r,
        })

    nc = _get_module()
    res = run_bass_kernel_spmd(
        nc, in_maps, core_ids=list(range(NCORES)), trace=TRACE
    )
    global LAST_RESULTS
    LAST_RESULTS = res
    t0_sum = 0.0
    lse_sum = 0.0
    for r in res.results:
        o = r["out"]
        t0_sum += float(o[0, 0])
        lse_sum += float(o[0, 1])
    val = t0_sum / N - (lse_sum / N - math.log(N))
    return np.float32(val)

